# revision 2
# baseline (speedup 1.0000x reference)
"""MultiHeadLatentAttention (MLA) Trainium2 kernel — 8-core SPMD, tensor-parallel over heads.

v2 restructure vs baseline:
  - qn/qp/kn/v intermediates stay RESIDENT IN SBUF between phases (the baseline
    round-tripped ~14MB/core through DRAM scratch between the Q/KV phases and
    attention).
  - The q-rmsnorm sum-of-squares is TOKEN-sharded (all 1536 ranks, fp8 DoubleRow,
    512-token shard) and fused into phase A; alpha = rsqrt(mean+eps) is computed
    on the producer and gathered with a tiny [1,512] AllGather.  This removes
    both per-batch AllReduces of the baseline.
  - kv latent + rope key ride ONE packed AllGather ([128, 5, 512]: 4 latent
    chunks + kr in chunk 4 rows 0-63) instead of two.
  - Attention issues NO Pool-engine ops: the softmax denominator partition
    reduce + broadcast are ones-matmuls on the tensor engine, so the AllToAll
    collectives (which occupy the Pool queue until completion) cannot stall
    attention of the other batch.
  - All front-phase matmuls use 512-wide free dims (full PSUM bank) to amortize
    instruction issue + LDWEIGHTS.
Per core c: owns heads 2c, 2c+1; token shard = batch c//4, tokens (c%4)*512.
"""

import math
import numpy as np

import concourse.bacc as bacc
import concourse.mybir as mybir
import concourse.tile as tile
from concourse.bass_utils import run_bass_kernel_spmd

F32 = mybir.dt.float32
F32R = mybir.dt.float32r
BF16 = mybir.dt.bfloat16
F8 = mybir.dt.float8e4
DRPM = mybir.MatmulPerfMode.DoubleRow

SXQ = 32.0            # fp8 scale for x
SWQ = 1024.0          # fp8 scale for wq_down

N_CORES = 8
HPC = 2               # heads per core
DIM = 2048
NH = 16
QR = 1536
KVR = 512
DN = 128
DR = 64
DV = 128
B = 2
S = 2048
T = B * S
EPS = 1e-6
SCALE = 1.0 / math.sqrt(DN + DR)
ROPE_THETA = 10000.0

TB = 512              # token block (front phases)
DCH = DIM // 128      # 16 contraction chunks
TSH = T // N_CORES    # 512-token shard per core
TW = S // N_CORES     # 256-token per-batch output shard per core
NG = QR // 128        # 12 rank groups for the ss pass

_BUILD_CACHE = {}


def _build_program(reps=1, ablate=""):
    """ablate: 'F' skips the front phases, 'A' attention, 'W' the wo stage."""
    key = ("nc", reps, ablate)
    if key in _BUILD_CACHE:
        return _BUILD_CACHE[key]
    nF = "F" not in ablate
    nA = "A" not in ablate
    nW = "W" not in ablate

    nc = bacc.Bacc(num_devices=N_CORES)

    # ---------------- DRAM I/O ----------------
    xT_d = nc.dram_tensor("xT", [B, DIM, S], BF16, kind="ExternalInput")
    xs_d = nc.dram_tensor("xs", [DIM, TSH], BF16, kind="ExternalInput")
    xqs_d = nc.dram_tensor("xqs", [DIM, TSH], F8, kind="ExternalInput")
    wqd8_d = nc.dram_tensor("wqd8", [DIM, QR], F8, kind="ExternalInput")
    wq_d = nc.dram_tensor("wq", [DIM, HPC * DN], BF16, kind="ExternalInput")
    wqp_d = nc.dram_tensor("wqp", [DIM, HPC * DR], BF16, kind="ExternalInput")
    wkvd_d = nc.dram_tensor("wkvd", [DIM, KVR], BF16, kind="ExternalInput")
    wkvu_d = nc.dram_tensor("wkvu", [KVR, HPC * (DN + DV)], BF16, kind="ExternalInput")
    wkr_d = nc.dram_tensor("wkr", [DIM, DR], BF16, kind="ExternalInput")
    wof_d = nc.dram_tensor("wof", [DV, NH, DIM], BF16, kind="ExternalInput")
    ctab_d = nc.dram_tensor("ctab", [128, S], F32, kind="ExternalInput")
    stab_d = nc.dram_tensor("stab", [128, S], F32, kind="ExternalInput")
    ctabs_d = nc.dram_tensor("ctabs", [64, TSH], F32, kind="ExternalInput")
    stabs_d = nc.dram_tensor("stabs", [64, TSH], F32, kind="ExternalInput")
    masks_d = nc.dram_tensor("masks", [128, 128], BF16, kind="ExternalInput")
    ident_d = nc.dram_tensor("ident", [128, 128], BF16, kind="ExternalInput")
    ones_d = nc.dram_tensor("ones", [128, 1], F32R, kind="ExternalInput")
    ones1_d = nc.dram_tensor("ones1", [1, 128], F32R, kind="ExternalInput")

    yT_d = nc.dram_tensor("yT", [B, DIM, TW], F32, kind="ExternalOutput")

    # ---------------- internal DRAM ----------------
    agin_d = nc.dram_tensor("agin", [128, 5, TSH], BF16)
    agout_d = nc.dram_tensor("agout", [N_CORES, 128, 5, TSH], BF16, addr_space="Shared")
    alin_d = nc.dram_tensor("alin", [1, TSH], F32)
    alout_d = nc.dram_tensor("alout", [N_CORES, 1, TSH], F32, addr_space="Shared")
    a2ai = [nc.dram_tensor(f"a2ai_{b}", [N_CORES, HPC * DV, TW], BF16) for b in range(B)]
    a2ao = [nc.dram_tensor(f"a2ao_{b}", [N_CORES, HPC * DV, TW], BF16) for b in range(B)]

    with tile.TileContext(nc) as tc:
        with tc.tile_pool(name="wpool", bufs=1) as wp:
            # resident weights / constants (loaded once)
            wq_t = wp.tile([128, DCH, HPC * DN], BF16, tag="wq")
            wqp_t = wp.tile([128, DCH, HPC * DR], BF16, tag="wqp")
            wkvu_t = wp.tile([128, KVR // 128, HPC * (DN + DV)], BF16, tag="wkvu")
            wkr_t = wp.tile([128, DCH, DR], BF16, tag="wkr")
            masks_t = wp.tile([128, 128], BF16, tag="masks")
            ident_t = wp.tile([128, 128], BF16, tag="ident")
            nc.sync.dma_start(ident_t[:], ident_d[:])
            nc.sync.dma_start(masks_t[:], masks_d[:])
            eps_t = wp.tile([128, 1], F32, tag="eps")
            nc.gpsimd.memset(eps_t[:], EPS)
            ones_t = wp.tile([128, 1], F32R, tag="ones")
            nc.sync.dma_start(ones_t[:], ones_d[:])
            ones1_t = wp.tile([1, 128], F32R, tag="ones1")
            nc.sync.dma_start(ones1_t[:], ones1_d[:])
            nc.sync.dma_start(wkvu_t[:], wkvu_d.ap().rearrange("(c p) m -> p c m", p=128))
            nc.sync.dma_start(wkr_t[:], wkr_d.ap().rearrange("(c p) m -> p c m", p=128))
            nc.sync.dma_start(wq_t[:], wq_d.ap().rearrange("(c p) m -> p c m", p=128))
            nc.sync.dma_start(wqp_t[:], wqp_d.ap().rearrange("(c p) m -> p c m", p=128))

            # persistent per-rep activations (SBUF-resident between phases)
            qn_sb = [[wp.tile([128, S], BF16, tag=f"qn{b}{h}", name=f"qn_sb{b}{h}")
                      for h in range(HPC)] for b in range(B)]
            qp_sb = [wp.tile([128, S], BF16, tag=f"qp{b}", name=f"qp_sb{b}") for b in range(B)]
            kn_sb = [[wp.tile([128, S], BF16, tag=f"kn{b}{h}", name=f"kn_sb{b}{h}")
                      for h in range(HPC)] for b in range(B)]
            v_sb = [[wp.tile([128, S // 128, DV], BF16, tag=f"v{b}{h}", name=f"v_sb{b}{h}")
                     for h in range(HPC)] for b in range(B)]

            for rep in range(reps):
                # ======================= PHASE A: kv/kr/ss on my token shard =======================
                with tc.tile_pool(name="fa", bufs=1) as fa, \
                     tc.tile_pool(name="pa", bufs=1, space="PSUM") as pa:
                    wkvd_t = fa.tile([128, DCH, KVR], BF16, tag="wkvd")
                    wqd8_t = fa.tile([128, DCH, QR], F8, tag="wqd8")
                    ctabs_t = fa.tile([64, TSH], F32, tag="ctabs")
                    stabs_t = fa.tile([64, TSH], F32, tag="stabs")
                    xs_t = fa.tile([128, DCH, TSH], BF16, tag="xs")
                    xqs_t = fa.tile([128, DCH, TSH], F8, tag="xqs")
                    if nF:
                        nc.sync.dma_start(xs_t[:], xs_d.ap().rearrange("(c p) t -> p c t", p=128))
                        nc.sync.dma_start(wkvd_t[:], wkvd_d.ap().rearrange("(c p) m -> p c m", p=128))
                        nc.sync.dma_start(ctabs_t[:], ctabs_d[:])
                        nc.sync.dma_start(stabs_t[:], stabs_d[:])
                        nc.sync.dma_start(xqs_t[:], xqs_d.ap().rearrange("(c p) t -> p c t", p=128))
                        nc.sync.dma_start(wqd8_t[:], wqd8_d.ap().rearrange("(c p) m -> p c m", p=128))

                    if nF:
                        # ---- kv_down (all 512 ranks) + beta ss ----
                        kvc = fa.tile([128, KVR // 128, TSH], F32, tag="kvc")
                        ssb = pa.tile([1, TSH], F32, tag="ssb")
                        sq_tiles = []
                        for rc in range(KVR // 128):
                            ps_kv = pa.tile([128, TSH], F32, tag="pkv", bufs=2)
                            for d in range(DCH):
                                nc.tensor.matmul(ps_kv[:], wkvd_t[:, d, rc * 128:(rc + 1) * 128],
                                                 xs_t[:, d, :], start=(d == 0), stop=(d == DCH - 1))
                            nc.vector.tensor_copy(kvc[:, rc, :], ps_kv[:])
                            sq_rc = fa.tile([128, TSH], F32R, tag="sq_rc", bufs=3)
                            nc.scalar.activation(sq_rc[:], ps_kv[:], mybir.ActivationFunctionType.Square)
                            sq_tiles.append(sq_rc)
                            if rc >= 1:
                                nc.tensor.matmul(ssb[:], ones_t[:, :], sq_tiles[rc - 1][:],
                                                 start=(rc == 1), stop=False)

                        # ---- k_rope on my shard ----
                        ps_kr = pa.tile([64, TSH], F32, tag="pkr")
                        for d in range(DCH):
                            nc.tensor.matmul(ps_kr[:], wkr_t[:, d, :], xs_t[:, d, :],
                                             start=(d == 0), stop=(d == DCH - 1))
                        nc.tensor.matmul(ssb[:], ones_t[:, :], sq_tiles[-1][:],
                                         start=False, stop=True)

                        # beta = 1/sqrt(mean + eps); normalize latent; pack + rope kr
                        brow = fa.tile([1, TSH], F32, tag="brow")
                        nc.scalar.activation(brow[:], ssb[:], mybir.ActivationFunctionType.Sqrt,
                                             scale=1.0 / KVR, bias=eps_t[0:1, :])
                        nc.vector.reciprocal(brow[:], brow[:])
                        bbc = fa.tile([128, TSH], F32, tag="bbc")
                        nc.gpsimd.partition_broadcast(bbc[:], brow[:])
                        kvs = fa.tile([128, KVR // 128, TSH], BF16, tag="kvs")
                        for rc in range(KVR // 128):
                            nc.vector.tensor_mul(kvs[:, rc, :], kvc[:, rc, :], bbc[:])
                        nc.sync.dma_start(agin_d.ap()[:, 0:4, :], kvs[:])
                        tmp = fa.tile([64, TSH], F32, tag="krtmp")
                        nc.vector.tensor_copy(tmp[0:32, :], ps_kr[32:64, :])
                        nc.vector.tensor_copy(tmp[32:64, :], ps_kr[0:32, :])
                        m1 = fa.tile([64, TSH], F32, tag="krm1")
                        nc.vector.tensor_mul(m1[:], ps_kr[:], ctabs_t[:])
                        nc.vector.tensor_mul(tmp[:], tmp[:], stabs_t[:])
                        krr = fa.tile([64, TSH], BF16, tag="krr")
                        nc.vector.tensor_add(krr[:], m1[:], tmp[:])
                        nc.sync.dma_start(agin_d.ap()[0:64, 4, :], krr[:])

                        nc.gpsimd.collective_compute(
                            "AllGather", mybir.AluOpType.bypass,
                            replica_groups=[list(range(N_CORES))],
                            ins=[agin_d[:]], outs=[agout_d[:]],
                        )

                        # ---- q-rmsnorm sum of squares, all 1536 ranks, my shard (fp8 DR) ----
                        ssq = pa.tile([1, TSH], F32, tag="ssq")
                        sqg_tiles = []
                        for g in range(NG):
                            ps_s = pa.tile([128, TSH], F32, tag="pss", bufs=2)
                            for d in range(DCH // 2):
                                nc.tensor.matmul(ps_s[:], wqd8_t[:, 2 * d:2 * d + 2, g * 128:(g + 1) * 128],
                                                 xqs_t[:, 2 * d:2 * d + 2, :],
                                                 start=(d == 0), stop=(d == DCH // 2 - 1),
                                                 perf_mode=DRPM)
                            sqg = fa.tile([128, TSH], F32R, tag="sqg", bufs=3)
                            nc.scalar.activation(sqg[:], ps_s[:], mybir.ActivationFunctionType.Square)
                            sqg_tiles.append(sqg)
                            if g >= 1:
                                nc.tensor.matmul(ssq[:], ones_t[:, :], sqg_tiles[g - 1][:],
                                                 start=(g == 1), stop=False)
                        nc.tensor.matmul(ssq[:], ones_t[:, :], sqg_tiles[-1][:],
                                         start=False, stop=True)
                        arow = fa.tile([1, TSH], F32, tag="arow")
                        nc.scalar.activation(arow[:], ssq[:], mybir.ActivationFunctionType.Sqrt,
                                             scale=1.0 / (QR * (SXQ * SWQ) ** 2), bias=eps_t[0:1, :])
                        nc.vector.reciprocal(arow[:], arow[:])
                        nc.sync.dma_start(alin_d[:], arow[:])
                        nc.gpsimd.collective_compute(
                            "AllGather", mybir.AluOpType.bypass,
                            replica_groups=[list(range(N_CORES))],
                            ins=[alin_d[:]], outs=[alout_d[:]],
                        )

                # apool opens early so the full wo ships while phase B computes
                with tc.tile_pool(name="apool", bufs=1) as ap:
                    wof_t = ap.tile([DV, NH, DIM], BF16, tag="wof")
                    if nW:
                        nc.sync.dma_start(wof_t[:], wof_d[:])

                    # ======================= PHASE B: Q path (all tokens, my 2 heads) =======================
                    with tc.tile_pool(name="fb", bufs=1) as fb, \
                         tc.tile_pool(name="pb", bufs=1, space="PSUM") as pb:
                        for b in range(B if nF else 0):
                            for j in range(S // TB):
                                t0 = j * TB
                                xt = fb.tile([128, DCH, TB], BF16, tag="xt", bufs=2)
                                nc.sync.dma_start(
                                    xt[:], xT_d.ap()[b, :, t0:t0 + TB].rearrange("(c p) t -> p c t", p=128))
                                ct = fb.tile([128, TB], F32, tag="ct", bufs=2)
                                st = fb.tile([128, TB], F32, tag="st", bufs=2)
                                nc.sync.dma_start(ct[:], ctab_d.ap()[:, t0:t0 + TB])
                                nc.sync.dma_start(st[:], stab_d.ap()[:, t0:t0 + TB])

                                for h in range(HPC):
                                    ps_qn = pb.tile([128, TB], F32, tag="pq", bufs=3)
                                    for d in range(DCH):
                                        nc.tensor.matmul(ps_qn[:], wq_t[:, d, h * DN:(h + 1) * DN],
                                                         xt[:, d, :], start=(d == 0), stop=(d == DCH - 1))
                                    nc.vector.tensor_copy(qn_sb[b][h][:, t0:t0 + TB], ps_qn[:])

                                ps_qp = pb.tile([128, TB], F32, tag="pq", bufs=3)
                                for d in range(DCH):
                                    nc.tensor.matmul(ps_qp[:], wqp_t[:, d, :], xt[:, d, :],
                                                     start=(d == 0), stop=(d == DCH - 1))
                                qtmp = fb.tile([128, TB], F32, tag="qptmp", bufs=2)
                                for h in range(HPC):
                                    o = h * 64
                                    nc.vector.tensor_copy(qtmp[o:o + 32, :], ps_qp[o + 32:o + 64, :])
                                    nc.vector.tensor_copy(qtmp[o + 32:o + 64, :], ps_qp[o:o + 32, :])
                                qm1 = fb.tile([128, TB], F32, tag="qpm1", bufs=2)
                                nc.vector.tensor_mul(qm1[:], ps_qp[:], ct[:])
                                nc.vector.tensor_mul(qtmp[:], qtmp[:], st[:])
                                nc.vector.tensor_add(qp_sb[b][:, t0:t0 + TB], qm1[:], qtmp[:])

                    # ======================= PHASE C: K/V up-projection from gathered latent =======================
                    with tc.tile_pool(name="fc", bufs=1) as fc, \
                         tc.tile_pool(name="pc", bufs=1, space="PSUM") as pc:
                        pend_v = None

                        def flush_v(pv):
                            stg_, b_, h_, soff_ = pv
                            for c2 in range(TSH // 128):
                                tps = pc.tile([128, 128], BF16, tag="ptp", bufs=3)
                                nc.tensor.transpose(tps[:], stg_[:, c2 * 128:(c2 + 1) * 128], ident_t[:])
                                nc.vector.tensor_copy(v_sb[b_][h_][:, soff_ // 128 + c2, :], tps[:])

                        for g in range(N_CORES if nF else 0):
                            b, soff = g // 4, (g % 4) * TSH
                            kvg_t = fc.tile([128, KVR // 128, TSH], BF16, tag="kvg", bufs=2)
                            nc.sync.dma_start(kvg_t[:], agout_d.ap()[g, :, 0:4, :])
                            for m in range(4):  # 0: K h0, 1: V h0, 2: K h1, 3: V h1
                                h, is_v = m // 2, m % 2
                                ps_up = pc.tile([128, TSH], F32, tag="pup", bufs=3)
                                for rc in range(KVR // 128):
                                    nc.tensor.matmul(ps_up[:], wkvu_t[:, rc, m * 128:(m + 1) * 128],
                                                     kvg_t[:, rc, :], start=(rc == 0), stop=(rc == 3))
                                if pend_v is not None:
                                    flush_v(pend_v)
                                    pend_v = None
                                if not is_v:
                                    nc.vector.tensor_copy(kn_sb[b][h][:, soff:soff + TSH], ps_up[:])
                                else:
                                    stg = fc.tile([128, TSH], BF16, tag="stg_up", bufs=3)
                                    nc.vector.tensor_copy(stg[:], ps_up[:])
                                    pend_v = (stg, b, h, soff)
                        if nF:
                            flush_v(pend_v)

                    # ======================= ATTENTION =======================
                    with tc.tile_pool(name="aps", bufs=1, space="PSUM") as app:
                        # alpha broadcast for both batches up front (Pool is free here;
                        # everything after runs on PE/ACT/DVE only, so the AllToAlls
                        # never gate attention)
                        abc_t = []
                        for b in range(B if nA else 0):
                            arow_t = ap.tile([1, S], F32, tag=f"arow{b}", name=f"arow{b}")
                            for ck in range(4):
                                nc.sync.dma_start(arow_t[0:1, ck * TSH:(ck + 1) * TSH],
                                                  alout_d.ap()[4 * b + ck])
                            abc = ap.tile([128, S], F32, tag=f"abc{b}", name=f"abc{b}")
                            nc.gpsimd.partition_broadcast(abc[:], arow_t[:])
                            abc_t.append(abc)

                        for b in range(B if nA else 0):
                            abc = abc_t[b]
                            kr_sb = ap.tile([64, S], BF16, tag="kr_sb", bufs=2)
                            for ck in range(4):
                                nc.sync.dma_start(kr_sb[:, ck * TSH:(ck + 1) * TSH],
                                                  agout_d.ap()[4 * b + ck, 0:64, 4, :])

                            out_sb = [ap.tile([128, S], BF16, tag=f"out{h}", bufs=2,
                                              name=f"out_sb{b}{h}") for h in range(HPC)]

                            for h in range(HPC):
                                for qt in range(4):
                                    q0 = qt * 512
                                    nkc = 4 * (qt + 1)
                                    qn_sc = ap.tile([128, 512], BF16, tag="qn_sc", bufs=2)
                                    nc.vector.tensor_mul(qn_sc[:], qn_sb[b][h][:, q0:q0 + 512],
                                                         abc[:, q0:q0 + 512])
                                    qp_sc = ap.tile([64, 512], BF16, tag="qp_sc", bufs=2)
                                    nc.vector.tensor_mul(qp_sc[:], qp_sb[b][h * 64:(h + 1) * 64, q0:q0 + 512],
                                                         abc[0:64, q0:q0 + 512])

                                    O = app.tile([128, 512], F32, tag="pO", bufs=2)
                                    l_acc = ap.tile([128, 512], F32R, tag="l_acc", bufs=2)

                                    def emit_scores(kc, kr_sb=kr_sb, qn_sc=qn_sc, qp_sc=qp_sc, qt=qt, b=b, h=h):
                                        k0 = kc * 128
                                        f0 = max(kc - 4 * qt, 0) * 128
                                        s_ps = app.tile([128, 512], F32, tag="ps_s", bufs=3)
                                        nc.tensor.matmul(s_ps[:, f0:], kn_sb[b][h][:, k0:k0 + 128],
                                                         qn_sc[:, f0:], start=True, stop=False)
                                        nc.tensor.matmul(s_ps[:, f0:], kr_sb[:, k0:k0 + 128],
                                                         qp_sc[:, f0:], start=False, stop=True)
                                        P = ap.tile([128, 512], BF16, tag="P", bufs=4)
                                        nc.scalar.activation(P[:, f0:], s_ps[:, f0:],
                                                             mybir.ActivationFunctionType.Exp,
                                                             scale=SCALE)
                                        if kc >= 4 * qt:
                                            nc.vector.tensor_mul(P[:, f0:f0 + 128], P[:, f0:f0 + 128],
                                                                 masks_t[:])
                                        return P, f0

                                    Pf = emit_scores(0)
                                    for kc in range(nkc):
                                        Pn = emit_scores(kc + 1) if kc + 1 < nkc else None
                                        P, f0 = Pf
                                        if kc == 0:
                                            nc.vector.tensor_copy(l_acc[:], P[:])
                                        else:
                                            nc.vector.tensor_add(l_acc[:, f0:], l_acc[:, f0:], P[:, f0:])
                                        nc.tensor.matmul(O[:, f0:], v_sb[b][h][:, kc, :], P[:, f0:],
                                                         start=(kc == 0), stop=(kc == nkc - 1))
                                        Pf = Pn
                                    # denominator: partition-reduce + broadcast via PE
                                    l_row = app.tile([1, 512], F32, tag="plr", bufs=2)
                                    nc.tensor.matmul(l_row[:], ones_t[:, :], l_acc[:],
                                                     start=True, stop=True)
                                    rrow = ap.tile([1, 512], F32R, tag="rrow", bufs=2)
                                    with nc.allow_low_precision(reason="softmax denom recip row, f32r for PE broadcast"):
                                        nc.vector.reciprocal(rrow[:], l_row[:])
                                    lbc = app.tile([128, 512], F32, tag="plb", bufs=1)
                                    nc.tensor.matmul(lbc[:], ones1_t[:, :], rrow[:],
                                                     start=True, stop=True)
                                    nc.vector.tensor_mul(out_sb[h][:, q0:q0 + 512], O[:], lbc[:])

                            # scatter this batch's heads to their token-owner cores
                            for d in range(N_CORES):
                                for h in range(HPC):
                                    nc.sync.dma_start(
                                        a2ai[b].ap()[d, h * DV:(h + 1) * DV, :],
                                        out_sb[h][:, d * TW:(d + 1) * TW])
                            nc.gpsimd.collective_compute(
                                "AllToAll", mybir.AluOpType.bypass,
                                replica_groups=[list(range(N_CORES))],
                                ins=[a2ai[b][:]], outs=[a2ao[b][:]],
                            )

                    # ======================= WO: my 256-token shard, all 16 heads =======================
                    with tc.tile_pool(name="pw", bufs=1, space="PSUM") as pw:
                        for b in range(B if nW else 0):
                            att_t = ap.tile([128, NH, TW], BF16, tag="att", bufs=2)
                            for s8 in range(N_CORES):
                                nc.sync.dma_start(
                                    att_t[:, HPC * s8:HPC * (s8 + 1), :],
                                    a2ao[b].ap()[s8].rearrange("(c p) t -> p c t", p=128))
                            for dm in range(DCH):
                                y_ps = pw.tile([128, TW], F32, tag="py", bufs=2)
                                for hc in range(NH):
                                    nc.tensor.matmul(y_ps[:], wof_t[:, hc, dm * 128:(dm + 1) * 128],
                                                     att_t[:, hc, :], start=(hc == 0), stop=(hc == NH - 1))
                                y_sb = ap.tile([128, TW], F32, tag="y_sb", bufs=3)
                                nc.vector.tensor_copy(y_sb[:], y_ps[:])
                                nc.sync.dma_start(yT_d.ap()[b, dm * 128:(dm + 1) * 128, :], y_sb[:])

                if ablate:
                    nc.all_engine_barrier()

    nc.finalize()
    _BUILD_CACHE[key] = nc
    return nc


def _host_inputs(x, wq_down, q_norm_w, wq_up, wq_rope, wkv_down, kv_norm_w, wkv_up, wk_rope, wo):
    """Build the 8 per-core input maps."""
    import ml_dtypes
    bf16 = ml_dtypes.bfloat16
    f8 = ml_dtypes.float8_e4m3    # TRN FP8_EXP4-compatible
    f32 = np.float32

    def q8(a, s):
        return np.ascontiguousarray(
            np.clip(np.asarray(a, f32) * s, -240.0, 240.0).astype(f8))

    x = np.asarray(x, f32)
    xT = np.ascontiguousarray(np.transpose(x, (0, 2, 1)))          # [B, DIM, S]
    xTb = xT.astype(bf16)

    p64 = np.concatenate([np.arange(0, DR, 2), np.arange(1, DR, 2)])  # deinterleave

    wq_down_n = (np.asarray(q_norm_w, f32)[:, None] * np.asarray(wq_down, f32))  # [QR, DIM]
    wkv_up_eff = np.asarray(wkv_up, f32) * np.asarray(kv_norm_w, f32)[None, :]   # [NH*(DN+DV), KVR]

    inv_freq = (1.0 / (ROPE_THETA ** (np.arange(0, DR, 2, dtype=np.float64) / DR)))  # [32]
    ang = np.arange(S, dtype=np.float64)[:, None] * inv_freq[None, :]                # [S, 32]
    cos_t, sin_t = np.cos(ang), np.sin(ang)
    C64 = np.concatenate([cos_t.T, cos_t.T], axis=0).astype(f32)                     # [64, S]
    S64 = np.concatenate([-sin_t.T, sin_t.T], axis=0).astype(f32)                    # [64, S]
    ctab = np.concatenate([C64, C64], axis=0)                                        # [128, S]
    stab = np.concatenate([S64, S64], axis=0)

    kr = np.arange(128)[:, None]
    qr = np.arange(128)[None, :]
    masks = (kr <= qr).astype(bf16)                                                  # [128, 128]

    ident = np.eye(128, dtype=bf16)
    wof = np.ascontiguousarray(
        np.asarray(wo, f32).reshape(DIM, NH, DV).transpose(2, 1, 0)).astype(bf16)    # [DV, NH, DIM]
    wqd8 = q8(np.asarray(wq_down, f32).T, SWQ)                                       # [DIM, QR]

    in_maps = []
    for c in range(N_CORES):
        h0, h1 = HPC * c, HPC * c + 1
        wq_blocks, wqp_blocks, wkvu_cols = [], [], []
        for h in (h0, h1):
            wq_blocks.append(np.asarray(wq_up, f32)[h * DN:(h + 1) * DN, :] @ wq_down_n)
            wr = np.asarray(wq_rope, f32)[h * DR:(h + 1) * DR, :][p64, :]
            wqp_blocks.append(wr @ wq_down_n)
            wkvu_cols.append(wkv_up_eff[h * (DN + DV): h * (DN + DV) + DN, :].T)      # K_h  [KVR, DN]
            wkvu_cols.append(wkv_up_eff[h * (DN + DV) + DN: (h + 1) * (DN + DV), :].T)  # V_h
        bA, sA = c // (N_CORES // B), (c % (N_CORES // B)) * TSH
        in_maps.append({
            "xT": xTb,
            "xs": np.ascontiguousarray(xTb[bA, :, sA:sA + TSH]),
            "xqs": q8(xT[bA, :, sA:sA + TSH], SXQ),
            "wqd8": wqd8,
            "wq": np.ascontiguousarray(np.concatenate(wq_blocks, axis=0).T).astype(bf16),
            "wqp": np.ascontiguousarray(np.concatenate(wqp_blocks, axis=0).T).astype(bf16),
            "wkvd": np.ascontiguousarray(np.asarray(wkv_down, f32).T).astype(bf16),
            "wkvu": np.ascontiguousarray(np.concatenate(wkvu_cols, axis=1)).astype(bf16),
            "wkr": np.ascontiguousarray(np.asarray(wk_rope, f32)[p64, :].T).astype(bf16),
            "wof": wof,
            "ctab": ctab,
            "stab": stab,
            "ctabs": np.ascontiguousarray(C64[:, sA:sA + TSH]),
            "stabs": np.ascontiguousarray(S64[:, sA:sA + TSH]),
            "masks": masks,
            "ident": ident,
            "ones": np.ones((128, 1), f32),
            "ones1": np.ones((1, 128), f32),
        })
    return in_maps


def kernel(**inputs) -> np.ndarray:
    nc = _build_program(1)
    in_maps = _host_inputs(**inputs)
    res = run_bass_kernel_spmd(nc, in_maps, core_ids=list(range(N_CORES)))
    yT = np.zeros((B, DIM, S), np.float32)
    for c in range(N_CORES):
        yT[:, :, c * TW:(c + 1) * TW] = res.results[c]["yT"]
    return np.ascontiguousarray(yT.transpose(0, 2, 1))
